# revision 25
# baseline (speedup 1.0000x reference)
"""ComplEx rhs-scoring kernel for Trainium2 (8 NeuronCores).

scores = Re(<lhs * rel, conj(all_ents)>) = q @ ent_emb.T
where q = [q_re, q_im] (complex product of gathered lhs/rel embeddings).

Strategy (tensor-parallel over candidates):
  - host: gather + complex product -> q [B, K] (tiny, exact fp32),
    transpose to qT [K, B]; transpose ent_emb -> eT [K, N]; split eT
    into 8 column slabs [K, N/8] (one per core); replicate qT.
  - device (per core): scores_slab[b, n] = sum_k qT[k, b] * eT[k, n]
    via PE matmuls: lhsT = qT k-tile [128, 128], rhs = eT chunk
    [128, CW], accumulate K/128 = 8 matmuls into PSUM fp32.
  - host: concat slabs along axis 1 -> [B, N] (upcast bf16 -> f32).

Timing structure (per core, bf16): 1600 matmuls (24x512 + 1x212
chunks, x8 b-tiles x8 k-tiles) at the N/2.4GHz+2.5ns streaming floor
= 337us; everything else (head DMA latency, PE clock-gate warmup,
output drain, framework pre/postamble) is overlap engineering around
that floor. bf16 (vs f32r) matters twice: FWL halves LDWEIGHTS so it
fully hides under the 211ns matmul stream (f32r measured 227ns/MM),
and input DMA halves.
"""

import os
import numpy as np

import concourse.bacc as bacc
import concourse.mybir as mybir
import concourse.tile as tile
from concourse.bass_utils import run_bass_kernel_spmd

N_CORES = 8
B = 1024          # batch (queries)
K = 1024          # contraction dim (2 * rank)
N_ENT = 100000    # candidates
NS = N_ENT // N_CORES  # per-core slab width (12500)
P = 128           # partitions
KT = K // P       # k tiles (8)
BT = B // P       # b tiles (8)
CW = 512          # rhs chunk width (one full PSUM bank)

_DT = {
    "bf16": mybir.dt.bfloat16,
    "f32r": mybir.dt.float32r,
    "f32": mybir.dt.float32,
}


def build_kernel(dt_name, ns=NS, cw=CW, b=B):
    dt_in = _DT[dt_name]
    f32 = mybir.dt.float32
    # bf16 path also writes bf16 output (host upcasts): halves the
    # writeback DMA traffic; rounding adds <0.1% error vs the 2% gate
    dt_out = dt_in if dt_name == "bf16" else f32
    nc = bacc.Bacc("TRN2", target_bir_lowering=False, debug=False)

    qT = nc.dram_tensor("qT", [K, b], dt_in, kind="ExternalInput")
    eT = nc.dram_tensor("eT", [K, ns], dt_in, kind="ExternalInput")
    # the last (remainder) chunk is written as a RAW tile dump to out2
    # (contiguous per partition -> coarse DMA descriptors, ~3x faster
    # completion than the strided [b, ns] write); the host interleaves
    # it back. Everything else goes to out.
    w_last = ns % cw if ns % cw else cw
    n_main = ns - w_last
    out = nc.dram_tensor("out", [b, n_main], dt_out, kind="ExternalOutput")
    bt_ = b // P
    out2 = nc.dram_tensor("out2", [P, bt_ * w_last], dt_out,
                          kind="ExternalOutput")
    out2_r = out2.rearrange("p (bt w) -> p bt w", bt=bt_)

    bt = b // P
    # remainder chunk goes LAST: its copies and writeback are ~2.4x
    # smaller than a full chunk's, so the post-stream drain (which is
    # serial: last matmul -> copy -> dma issue -> transfer -> teardown)
    # trails the last matmul by ~1.2us instead of ~2.7us
    widths = [cw] * (ns // cw)
    if ns % cw:
        widths = widths + [ns % cw]
    offs = [sum(widths[:i]) for i in range(len(widths))]
    n_chunks = len(widths)
    w0 = widths[0]

    # 3D-AP views: put the 128-partition dim first, keep k/b tile index
    # as a middle dim so a whole chunk moves in ONE dma_start (the sync
    # engine's ~0.7us per-issue cost is the scarce resource here).
    eT_r = eT.rearrange("(kt p) n -> p kt n", p=P)    # [128, KT, ns]
    qT_r = qT.rearrange("(kt p) b -> p kt b", p=P)    # [128, KT, b]
    out_r = out.rearrange("(bt p) n -> p bt n", p=P)  # [128, bt, n_main]

    with tile.TileContext(nc) as tc:
        with (
            tc.tile_pool(name="qpool", bufs=1) as qpool,
            tc.tile_pool(name="epool", bufs=4) as epool,
            tc.tile_pool(name="pspool", bufs=8, space="PSUM") as pspool,
            tc.tile_pool(name="opool", bufs=2) as opool,
        ):
            et0 = epool.tile([P, KT * w0], dt_in, tag="et")
            qsb = qpool.tile([P, KT * b], dt_in)
            qsb_r = qsb.rearrange("p (kt b) -> p kt b", kt=KT)

            kh = KT // 2

            # PE warmup (HAM clock-gate): a few dummy matmuls on a
            # memset tile keep the PE busy from right after the
            # preamble until the first real data lands; chunk 0's own
            # cold-rate matmuls then finish the ~3.4us warm window
            # doing real work. gpsimd executes the memset because its
            # queue frees first after the framework preamble.
            ww = 250
            warm = qpool.tile([P, ww], mybir.dt.bfloat16, name="warm")
            nc.gpsimd.memset(warm[:], 0.0)
            ps_w = pspool.tile([P, ww], f32, tag="ps", name="ps_warm")
            for _ in range(12):
                nc.tensor.matmul(ps_w[:], warm[:, 0:P], warm[:],
                                 start=True, stop=True)

            # head loads, k-granular so compute can start after just
            # q[k0]+et0[k0] (~0.4MB) instead of the full 3MB. The
            # ~0.7us per-dma_start issue cost is per-QUEUE, so the
            # first k-slice fans across the three DMA-capable queues
            # (sync+gpsimd for q halves, scalar for et0); remaining
            # k-slices alternate sync (q) / scalar (et0).
            bh = b // 2
            nc.sync.dma_start(qsb_r[:, 0, 0:bh], qT_r[:, 0, 0:bh])
            nc.gpsimd.dma_start(qsb_r[:, 0, bh:b], qT_r[:, 0, bh:b])
            nc.scalar.dma_start(et0[:, 0:w0], eT[0:P, 0:w0])
            for k in range(1, KT):
                nc.sync.dma_start(qsb_r[:, k, :], qT_r[:, k, :])
                nc.scalar.dma_start(
                    et0[:, k * w0:(k + 1) * w0],
                    eT[k * P:(k + 1) * P, 0:w0],
                )

            for c in range(n_chunks):
                w = widths[c]
                off = offs[c]
                if c == 0:
                    et = et0
                else:
                    et = epool.tile([P, KT * w], dt_in, tag="et", name=f"et{c}")
                    et_v = et.rearrange("p (kt w) -> p kt w", kt=KT)
                    # k-halves split across sync and gpsimd: halves the
                    # per-queue issue backlog right after the head burst
                    for j, eng in enumerate((nc.sync, nc.gpsimd)):
                        eng.dma_start(
                            et_v[:, j * kh:(j + 1) * kh, :],
                            eT_r[:, j * kh:(j + 1) * kh, off:off + w],
                        )
                ot = opool.tile([P, bt * w], dt_out, tag="ot", name=f"ot{c}")
                ot_h = ot.rearrange("p (bt w) -> p bt w", bt=bt)

                if c == 0:
                    # k-major: all 8 b-tiles accumulate in 8 PSUM banks
                    # simultaneously; each k-step needs only q[k]+et0[k]
                    # so compute starts ~7us in, right off the first DMA
                    pss = [pspool.tile([P, w], f32, tag="ps",
                                       name=f"ps0_{bi}") for bi in range(bt)]
                    for k in range(KT):
                        for bi in range(bt):
                            nc.tensor.matmul(
                                pss[bi][:],
                                qsb[:, k * b + bi * P:k * b + (bi + 1) * P],
                                et[:, k * w:(k + 1) * w],
                                start=(k == 0),
                                stop=(k == KT - 1),
                            )
                    for bi in range(bt):
                        if bi % 2 == 0:
                            nc.vector.tensor_copy(
                                ot[:, bi * w:(bi + 1) * w], pss[bi][:])
                        else:
                            nc.scalar.copy(
                                ot[:, bi * w:(bi + 1) * w], pss[bi][:])
                else:
                    # bi-major: one PSUM bank at a time, k inner
                    for bi in range(bt):
                        ps = pspool.tile([P, w], f32, tag="ps", name="ps")
                        for k in range(KT):
                            nc.tensor.matmul(
                                ps[:],
                                qsb[:, k * b + bi * P:k * b + (bi + 1) * P],
                                et[:, k * w:(k + 1) * w],
                                start=(k == 0),
                                stop=(k == KT - 1),
                            )
                        if bi % 2 == 0:
                            nc.vector.tensor_copy(
                                ot[:, bi * w:(bi + 1) * w], ps[:])
                        else:
                            nc.scalar.copy(
                                ot[:, bi * w:(bi + 1) * w], ps[:])

                # writeback rides the scalar queue: it stays off the
                # sync queue (whose head-of-line order gates entity
                # prefetch) and off gpsimd (whose software DRAIN costs
                # ~2.3us + 29ns/DMA at teardown; the scalar drain is
                # ~0.5us). One coarse DMA per chunk, except the last
                # chunk goes out in bi-pair slices so the final
                # transfer trails the last matmul by a pair only.
                if c < n_chunks - 1:
                    nc.scalar.dma_start(out_r[:, :, off:off + w],
                                        ot_h[:, :, :])
                else:
                    # two 4-bi raw-dump slices: the bi0-3 DMA issues
                    # while bi4-7 matmuls still run, so only the bi4-7
                    # slice (copy + ~0.6us issue + small transfer)
                    # trails the last MM. More slices would lose: each
                    # dma_start costs ~0.6us of serial issue time.
                    hb = bt // 2
                    nc.scalar.dma_start(
                        out2_r[:, 0:hb, :], ot_h[:, 0:hb, :])
                    nc.scalar.dma_start(
                        out2_r[:, hb:bt, :], ot_h[:, hb:bt, :])
    nc.compile()
    return nc


def _prep_inputs(x, ent_emb, rel_emb, dt_name):
    x = np.asarray(x)
    ent_emb = np.asarray(ent_emb, dtype=np.float32)
    rel_emb = np.asarray(rel_emb, dtype=np.float32)
    r = ent_emb.shape[1] // 2
    lhs = ent_emb[x[:, 0]]
    rel = rel_emb[x[:, 1]]
    lre, lim = lhs[:, :r], lhs[:, r:]
    rre, rim = rel[:, :r], rel[:, r:]
    q = np.empty((x.shape[0], 2 * r), np.float32)
    q[:, :r] = lre * rre - lim * rim
    q[:, r:] = lre * rim + lim * rre

    if dt_name == "bf16":
        import ml_dtypes
        np_dt = ml_dtypes.bfloat16
    else:
        np_dt = np.float32

    qT = np.ascontiguousarray(q.T).astype(np_dt)           # [K, B]
    eT = np.ascontiguousarray(ent_emb.T).astype(np_dt)     # [K, N]
    in_maps = [
        {"qT": qT, "eT": np.ascontiguousarray(eT[:, i * NS:(i + 1) * NS])}
        for i in range(N_CORES)
    ]
    return in_maps


def run(x, ent_emb, rel_emb, dt_name=None, trace=False, **spmd_kwargs):
    dt_name = dt_name or os.environ.get("KERNEL_DT", "bf16")
    nc = build_kernel(dt_name)
    in_maps = _prep_inputs(x, ent_emb, rel_emb, dt_name)
    res = run_bass_kernel_spmd(
        nc, in_maps, list(range(N_CORES)), trace=trace, **spmd_kwargs
    )
    w_last = NS % CW if NS % CW else CW
    outs = []
    for i in range(N_CORES):
        main = np.asarray(res.results[i]["out"], dtype=np.float32)
        tail = np.asarray(res.results[i]["out2"], dtype=np.float32)
        # out2 is a raw [P, bt*w_last] tile dump; row bt*P+p of the
        # slab is tail[p, bt*w_last:(bt+1)*w_last]
        tail = tail.reshape(P, BT, w_last).transpose(1, 0, 2).reshape(
            B, w_last)
        outs.append(main)
        outs.append(tail)
    return np.concatenate(outs, axis=1), res


def kernel(x, ent_emb, rel_emb):
    out, _ = run(x, ent_emb, rel_emb)
    return out


# revision 27
# speedup vs baseline: 1.0183x; 1.0183x over previous
"""ComplEx rhs-scoring kernel for Trainium2 (8 NeuronCores).

scores = Re(<lhs * rel, conj(all_ents)>) = q @ ent_emb.T
where q = [q_re, q_im] (complex product of gathered lhs/rel embeddings).

Strategy (tensor-parallel over candidates):
  - host: gather + complex product -> q [B, K] (tiny, exact fp32),
    transpose to qT [K, B]; transpose ent_emb -> eT [K, N]; split eT
    into 8 column slabs [K, N/8] (one per core); replicate qT.
  - device (per core): scores_slab[b, n] = sum_k qT[k, b] * eT[k, n]
    via PE matmuls: lhsT = qT k-tile [128, 128], rhs = eT chunk
    [128, CW], accumulate K/128 = 8 matmuls into PSUM fp32.
  - host: concat slabs along axis 1 -> [B, N] (upcast bf16 -> f32).

Timing structure (per core, bf16): 1600 matmuls (24x512 + 1x212
chunks, x8 b-tiles x8 k-tiles) at the N/2.4GHz+2.5ns streaming floor
= 337us; everything else (head DMA latency, PE clock-gate warmup,
output drain, framework pre/postamble) is overlap engineering around
that floor. bf16 (vs f32r) matters twice: FWL halves LDWEIGHTS so it
fully hides under the 211ns matmul stream (f32r measured 227ns/MM),
and input DMA halves.
"""

import os
import numpy as np

import concourse.bacc as bacc
import concourse.mybir as mybir
import concourse.tile as tile
from concourse.bass_utils import run_bass_kernel_spmd

N_CORES = 8
B = 1024          # batch (queries)
K = 1024          # contraction dim (2 * rank)
N_ENT = 100000    # candidates
NS = N_ENT // N_CORES  # per-core slab width (12500)
P = 128           # partitions
KT = K // P       # k tiles (8)
BT = B // P       # b tiles (8)
CW = 512          # rhs chunk width (one full PSUM bank)

_DT = {
    "bf16": mybir.dt.bfloat16,
    "f32r": mybir.dt.float32r,
    "f32": mybir.dt.float32,
}


def build_kernel(dt_name, ns=NS, cw=CW, b=B):
    dt_in = _DT[dt_name]
    f32 = mybir.dt.float32
    # bf16 path also writes bf16 output (host upcasts): halves the
    # writeback DMA traffic; rounding adds <0.1% error vs the 2% gate
    dt_out = dt_in if dt_name == "bf16" else f32
    nc = bacc.Bacc("TRN2", target_bir_lowering=False, debug=False)

    qT = nc.dram_tensor("qT", [K, b], dt_in, kind="ExternalInput")
    eT = nc.dram_tensor("eT", [K, ns], dt_in, kind="ExternalInput")
    # the last (remainder) chunk is written as a RAW tile dump to out2
    # (contiguous per partition -> coarse DMA descriptors, ~3x faster
    # completion than the strided [b, ns] write); the host interleaves
    # it back. Everything else goes to out.
    w_last = ns % cw if ns % cw else cw
    n_main = ns - w_last
    out = nc.dram_tensor("out", [b, n_main], dt_out, kind="ExternalOutput")
    bt_ = b // P
    out2 = nc.dram_tensor("out2", [P, bt_ * w_last], dt_out,
                          kind="ExternalOutput")
    out2_r = out2.rearrange("p (bt w) -> p bt w", bt=bt_)

    bt = b // P
    # remainder chunk goes LAST: its copies and writeback are ~2.4x
    # smaller than a full chunk's, so the post-stream drain (which is
    # serial: last matmul -> copy -> dma issue -> transfer -> teardown)
    # trails the last matmul by ~1.2us instead of ~2.7us
    widths = [cw] * (ns // cw)
    if ns % cw:
        widths = widths + [ns % cw]
    offs = [sum(widths[:i]) for i in range(len(widths))]
    n_chunks = len(widths)
    w0 = widths[0]

    # 3D-AP views: put the 128-partition dim first, keep k/b tile index
    # as a middle dim so a whole chunk moves in ONE dma_start (the sync
    # engine's ~0.7us per-issue cost is the scarce resource here).
    eT_r = eT.rearrange("(kt p) n -> p kt n", p=P)    # [128, KT, ns]
    qT_r = qT.rearrange("(kt p) b -> p kt b", p=P)    # [128, KT, b]
    out_r = out.rearrange("(bt p) n -> p bt n", p=P)  # [128, bt, n_main]

    with tile.TileContext(nc) as tc:
        with (
            tc.tile_pool(name="qpool", bufs=1) as qpool,
            tc.tile_pool(name="epool", bufs=4) as epool,
            tc.tile_pool(name="pspool", bufs=8, space="PSUM") as pspool,
            tc.tile_pool(name="opool", bufs=2) as opool,
        ):
            et0 = epool.tile([P, KT * w0], dt_in, tag="et")
            qsb = qpool.tile([P, KT * b], dt_in)
            qsb_r = qsb.rearrange("p (kt b) -> p kt b", kt=KT)

            kh = KT // 2

            # PE warmup (HAM clock-gate): a few dummy matmuls on a
            # memset tile keep the PE busy from right after the
            # preamble until the first real data lands; chunk 0's own
            # cold-rate matmuls then finish the ~3.4us warm window
            # doing real work. gpsimd executes the memset because its
            # queue frees first after the framework preamble.
            ww = 250
            warm = qpool.tile([P, ww], mybir.dt.bfloat16, name="warm")
            nc.gpsimd.memset(warm[:], 0.0)
            ps_w = pspool.tile([P, ww], f32, tag="ps", name="ps_warm")
            for _ in range(14):
                nc.tensor.matmul(ps_w[:], warm[:, 0:P], warm[:],
                                 start=True, stop=True)

            # head loads, k-granular so compute can start after just
            # q[k0]+et0[k0] (~0.4MB) instead of the full 3MB. The
            # ~0.7us per-dma_start issue cost is per-QUEUE, so the
            # first k-slice fans across the three DMA-capable queues
            # (sync+gpsimd for q halves, scalar for et0); remaining
            # k-slices alternate sync (q) / scalar (et0).
            bh = b // 2
            nc.sync.dma_start(qsb_r[:, 0, 0:bh], qT_r[:, 0, 0:bh])
            nc.gpsimd.dma_start(qsb_r[:, 0, bh:b], qT_r[:, 0, bh:b])
            nc.scalar.dma_start(et0[:, 0:w0], eT[0:P, 0:w0])
            for k in range(1, KT):
                nc.sync.dma_start(qsb_r[:, k, :], qT_r[:, k, :])
                nc.scalar.dma_start(
                    et0[:, k * w0:(k + 1) * w0],
                    eT[k * P:(k + 1) * P, 0:w0],
                )

            for c in range(n_chunks):
                w = widths[c]
                off = offs[c]
                if c == 0:
                    et = et0
                else:
                    et = epool.tile([P, KT * w], dt_in, tag="et", name=f"et{c}")
                    et_v = et.rearrange("p (kt w) -> p kt w", kt=KT)
                    # both k-halves stay on sync: queue order naturally
                    # deprioritizes this prefetch behind the head's
                    # critical q loads (splitting onto gpsimd made the
                    # prefetch compete with the head and stalled chunk 0)
                    for j in range(2):
                        nc.sync.dma_start(
                            et_v[:, j * kh:(j + 1) * kh, :],
                            eT_r[:, j * kh:(j + 1) * kh, off:off + w],
                        )
                ot = opool.tile([P, bt * w], dt_out, tag="ot", name=f"ot{c}")
                ot_h = ot.rearrange("p (bt w) -> p bt w", bt=bt)

                if c == 0:
                    # k-major: all 8 b-tiles accumulate in 8 PSUM banks
                    # simultaneously; each k-step needs only q[k]+et0[k]
                    # so compute starts ~7us in, right off the first DMA
                    pss = [pspool.tile([P, w], f32, tag="ps",
                                       name=f"ps0_{bi}") for bi in range(bt)]
                    for k in range(KT):
                        for bi in range(bt):
                            nc.tensor.matmul(
                                pss[bi][:],
                                qsb[:, k * b + bi * P:k * b + (bi + 1) * P],
                                et[:, k * w:(k + 1) * w],
                                start=(k == 0),
                                stop=(k == KT - 1),
                            )
                    for bi in range(bt):
                        if bi % 2 == 0:
                            nc.vector.tensor_copy(
                                ot[:, bi * w:(bi + 1) * w], pss[bi][:])
                        else:
                            nc.scalar.copy(
                                ot[:, bi * w:(bi + 1) * w], pss[bi][:])
                else:
                    # bi-major: one PSUM bank at a time, k inner
                    for bi in range(bt):
                        ps = pspool.tile([P, w], f32, tag="ps", name="ps")
                        for k in range(KT):
                            nc.tensor.matmul(
                                ps[:],
                                qsb[:, k * b + bi * P:k * b + (bi + 1) * P],
                                et[:, k * w:(k + 1) * w],
                                start=(k == 0),
                                stop=(k == KT - 1),
                            )
                        if bi % 2 == 0:
                            nc.vector.tensor_copy(
                                ot[:, bi * w:(bi + 1) * w], ps[:])
                        else:
                            nc.scalar.copy(
                                ot[:, bi * w:(bi + 1) * w], ps[:])

                # writeback rides the scalar queue: it stays off the
                # sync queue (whose head-of-line order gates entity
                # prefetch) and off gpsimd (whose software DRAIN costs
                # ~2.3us + 29ns/DMA at teardown; the scalar drain is
                # ~0.5us). One coarse DMA per chunk, except the last
                # chunk goes out in bi-pair slices so the final
                # transfer trails the last matmul by a pair only.
                if c < n_chunks - 1:
                    nc.scalar.dma_start(out_r[:, :, off:off + w],
                                        ot_h[:, :, :])
                else:
                    # two 4-bi raw-dump slices: the bi0-3 DMA issues
                    # while bi4-7 matmuls still run, so only the bi4-7
                    # slice (copy + ~0.6us issue + small transfer)
                    # trails the last MM. More slices would lose: each
                    # dma_start costs ~0.6us of serial issue time.
                    hb = bt // 2
                    nc.scalar.dma_start(
                        out2_r[:, 0:hb, :], ot_h[:, 0:hb, :])
                    nc.scalar.dma_start(
                        out2_r[:, hb:bt, :], ot_h[:, hb:bt, :])
    nc.compile()
    return nc


def _prep_inputs(x, ent_emb, rel_emb, dt_name):
    x = np.asarray(x)
    ent_emb = np.asarray(ent_emb, dtype=np.float32)
    rel_emb = np.asarray(rel_emb, dtype=np.float32)
    r = ent_emb.shape[1] // 2
    lhs = ent_emb[x[:, 0]]
    rel = rel_emb[x[:, 1]]
    lre, lim = lhs[:, :r], lhs[:, r:]
    rre, rim = rel[:, :r], rel[:, r:]
    q = np.empty((x.shape[0], 2 * r), np.float32)
    q[:, :r] = lre * rre - lim * rim
    q[:, r:] = lre * rim + lim * rre

    if dt_name == "bf16":
        import ml_dtypes
        np_dt = ml_dtypes.bfloat16
    else:
        np_dt = np.float32

    qT = np.ascontiguousarray(q.T).astype(np_dt)           # [K, B]
    eT = np.ascontiguousarray(ent_emb.T).astype(np_dt)     # [K, N]
    in_maps = [
        {"qT": qT, "eT": np.ascontiguousarray(eT[:, i * NS:(i + 1) * NS])}
        for i in range(N_CORES)
    ]
    return in_maps


def run(x, ent_emb, rel_emb, dt_name=None, trace=False, **spmd_kwargs):
    dt_name = dt_name or os.environ.get("KERNEL_DT", "bf16")
    nc = build_kernel(dt_name)
    in_maps = _prep_inputs(x, ent_emb, rel_emb, dt_name)
    res = run_bass_kernel_spmd(
        nc, in_maps, list(range(N_CORES)), trace=trace, **spmd_kwargs
    )
    w_last = NS % CW if NS % CW else CW
    outs = []
    for i in range(N_CORES):
        main = np.asarray(res.results[i]["out"], dtype=np.float32)
        tail = np.asarray(res.results[i]["out2"], dtype=np.float32)
        # out2 is a raw [P, bt*w_last] tile dump; row bt*P+p of the
        # slab is tail[p, bt*w_last:(bt+1)*w_last]
        tail = tail.reshape(P, BT, w_last).transpose(1, 0, 2).reshape(
            B, w_last)
        outs.append(main)
        outs.append(tail)
    return np.concatenate(outs, axis=1), res


def kernel(x, ent_emb, rel_emb):
    out, _ = run(x, ent_emb, rel_emb)
    return out
